# revision 21
# baseline (speedup 1.0000x reference)
"""Trainium2 Bass kernel for nn_Decoder (MLP -> inverse token embedding ->
overlap-add -> channel-merge conv), data-parallel over batch on 8 NeuronCores.

Self-contained: hardcodes shapes; host-side numpy folds everything after the
first Linear+ReLU into per-channel fused matrices G (W2 -> Winv -> overlap-add
normalization -> 3-tap channel conv), and pre-transposes x to feature-major
[TC, E, C*TL*BL] fp16 so the device needs NO transposes of x at all:

    xT[e, tok] --matmul W1T--> h[Hc,tok] in PSUM
    --ACT/DVE relu+bias--> hT in SBUF --matmul G (accum over c,Hc)--> v[66,tok]
    --PE transpose--> vT[b,66] --strided GpSimd adds (overlap-add)--> y[b,1056]

Sharding: batch 1024 -> 8 cores x 128.
"""

import numpy as np

import concourse.bacc as bacc
import concourse.mybir as mybir
from concourse.bass_utils import run_bass_kernel_spmd
from concourse.tile import TileContext

# problem shapes (hardcoded per contract)
B, C, T, E, H = 1024, 8, 32, 128, 256
SEG_LEN, SIG_LEN, NUM_SEG, STEP = 64, 1056, 32, 32
N_CORES = 8
BL = B // N_CORES          # local batch per core = 128
HC = H // 128              # H chunks = 2
TC = 8                     # t-chunks
TL = T // TC               # t per chunk = 4
CW = TL * BL               # tokens per (c, chunk) = 512
XW = C * CW                # tokens per chunk = 4096
FD = mybir.dt.float32
FR = mybir.dt.float32r     # fp32 storage, FP22 multiply
FH = mybir.dt.float16

_CACHE = {}


def _host_prep(W1, b1, W2, b2, Winv, binv, Wconv, bconv):
    """Fold W2/Winv/normalization/conv into G [3var][C][H,66] and bias B[1056]."""
    counter = np.zeros(SIG_LEN, np.float64)
    for t in range(NUM_SEG):
        counter[t * STEP: t * STEP + SEG_LEN] += 1.0
    n = 1.0 / counter

    F = Winv.astype(np.float64) @ W2.astype(np.float64)          # [64, H]
    binv2 = Winv.astype(np.float64) @ b2.astype(np.float64) + binv.astype(np.float64)
    Wc = Wconv[0].astype(np.float64)                             # [C, 3]

    def n_of(var, s):
        if var == 0:
            return n[s]
        if var == 2:
            return n[992 + s]
        return 0.5

    G = np.zeros((3, C, H, 66), np.float64)
    for var in range(3):
        for c in range(C):
            for m_idx in range(66):
                for k in range(3):
                    s = m_idx + k - 2
                    if 0 <= s < SEG_LEN:
                        G[var, c, :, m_idx] += Wc[c, k] * n_of(var, s) * F[s, :]

    sig_b = np.zeros(SIG_LEN, np.float64)
    for t in range(NUM_SEG):
        sig_b[t * STEP: t * STEP + SEG_LEN] += binv2
    sig_b *= n
    Bvec = np.full(SIG_LEN, float(np.asarray(bconv).reshape(-1)[0]), np.float64)
    q = np.arange(SIG_LEN)
    for k in range(3):
        qq = q + k - 1
        valid = (qq >= 0) & (qq < SIG_LEN)
        for c in range(C):
            Bvec[valid] += Wc[c, k] * sig_b[qq[valid]]
    return G.astype(np.float32), Bvec.astype(np.float32)


def _g_col(hc, c, var):
    """Column offset of G slice (hc, c, var) inside g_sb [128, 2*8*3*66]."""
    return ((hc * C + c) * 3 + var) * 66


def _chunk_ranges(tcix):
    # column ranges with uniform G variant; cols = tl*128 + b
    if tcix == 0:
        return [(0, 128, 0), (128, 512, 1)]       # t=0 -> var 0
    if tcix == TC - 1:
        return [(0, 384, 1), (384, 512, 2)]       # t=31 -> var 2
    return [(0, 512, 1)]


def _build_bass():
    nc = bacc.Bacc("TRN2")

    # host pre-transposed: x[tc, e, c*CW + tl*BL + b]  (feature-major)
    x = nc.dram_tensor("x", [TC, E, XW], FH, kind="ExternalInput")
    w1t = nc.dram_tensor("w1t", [E, H], FH, kind="ExternalInput")
    b1c = nc.dram_tensor("b1c", [128, HC], FD, kind="ExternalInput")
    g = nc.dram_tensor("g", [128, HC * C * 3 * 66], FH, kind="ExternalInput")
    brep = nc.dram_tensor("brep", [BL, SIG_LEN], FD, kind="ExternalInput")
    ident = nc.dram_tensor("ident", [128, 128], FR, kind="ExternalInput")
    y = nc.dram_tensor("y", [BL, SIG_LEN], FD, kind="ExternalOutput")

    with TileContext(nc) as tc:
        with (
            tc.tile_pool(name="consts", bufs=1) as consts,
            tc.tile_pool(name="xt", bufs=3) as xt_pool,
            tc.tile_pool(name="ht", bufs=2) as ht_pool,
            tc.tile_pool(name="vsb", bufs=3) as vsb_pool,
            tc.tile_pool(name="big", bufs=1) as big_pool,
            tc.tile_pool(name="h_ps", bufs=5, space="PSUM") as hps_pool,
            tc.tile_pool(name="v_ps", bufs=2, space="PSUM") as vps_pool,
            tc.tile_pool(name="pe_out", bufs=1, space="PSUM") as peout_pool,
        ):
            w1t_sb = consts.tile([E, H], FH)
            b1c_sb = consts.tile([128, HC], FD)
            g_sb = consts.tile([128, HC * C * 3 * 66], FH)
            ident_sb = consts.tile([128, 128], FR)
            brep_sb = big_pool.tile([BL, SIG_LEN], FD)

            V_sb = big_pool.tile([BL, T * 66], FD)      # v transposed: [b, t*66+m]
            y_sb = big_pool.tile([BL, SIG_LEN], FD)

            xt_tiles = {}

            def emit_x_load(tcix, split_first=False):
                t = xt_pool.tile([E, XW], FH, tag="xt", name=f"xt_{tcix}")
                if split_first:
                    # first chunk: land early channels quickly so PE starts sooner
                    nc.sync.dma_start(out=t[:, 0:2 * CW], in_=x[tcix, :, 0:2 * CW])
                    nc.sync.dma_start(out=t[:, 2 * CW:4 * CW],
                                      in_=x[tcix, :, 2 * CW:4 * CW])
                    nc.sync.dma_start(out=t[:, 4 * CW:], in_=x[tcix, :, 4 * CW:])
                else:
                    nc.sync.dma_start(out=t[:], in_=x[tcix])
                xt_tiles[tcix] = t

            # greedy ACT/DVE load balancer for PSUM->SBUF copies and relus
            eng_busy = {"act": 0.0, "dve": 0.0}

            def pick_engine(fd):
                ca = (172 + fd) / 1.2
                cd = (120 + fd) / 0.96
                if eng_busy["act"] + ca <= eng_busy["dve"] + cd:
                    eng_busy["act"] += ca
                    return "act"
                eng_busy["dve"] += cd
                return "dve"

            def bal_copy(out, in_, fd):
                if pick_engine(fd) == "act":
                    nc.scalar.copy(out=out, in_=in_)
                else:
                    nc.vector.tensor_copy(out=out, in_=in_)

            ht_tiles = {}

            def emit_mlp1_half(tcix, hc, cps):
                """one matmul + one relu drain per channel; 5 PSUM bufs keep
                the PE ahead of the ACT/DVE drain latency."""
                xt = xt_tiles[tcix]
                ht = ht_tiles[tcix]
                for cp in cps:
                    for half in range(2):
                        c = 2 * cp + half
                        h_ps = hps_pool.tile([128, CW], FD, tag="h_ps",
                                             name=f"h_ps_{tcix}_{hc}_{c}")
                        nc.tensor.matmul(
                            h_ps[:],
                            w1t_sb[:, hc * 128:(hc + 1) * 128],
                            xt[:, c * CW:(c + 1) * CW],
                            start=True, stop=True,
                        )
                        dst = ht[(cp, hc)][:, half * CW:(half + 1) * CW]
                        if pick_engine(CW) == "act":
                            nc.scalar.activation(
                                dst, h_ps[:],
                                mybir.ActivationFunctionType.Relu,
                                bias=b1c_sb[:, hc:hc + 1], scale=1.0,
                            )
                        else:
                            nc.vector.tensor_scalar(
                                dst, h_ps[:],
                                b1c_sb[:, hc:hc + 1], 0.0,
                                mybir.AluOpType.add, mybir.AluOpType.max,
                            )

            def emit_fused_half(tcix, v_tiles, hc):
                """fused G matmuls (one hc) accumulating into v_tiles ranges."""
                ht = ht_tiles[tcix]
                for c in range(C):
                    i = hc * C + c
                    hsrc = ht[(c // 2, hc)]
                    off = (c % 2) * CW
                    for (lo, hi, var, v_ps) in v_tiles:
                        nc.tensor.matmul(
                            v_ps[:, lo:hi],
                            g_sb[:, _g_col(hc, c, var):_g_col(hc, c, var) + 66],
                            hsrc[:, off + lo:off + hi],
                            start=(i == 0), stop=(i == HC * C - 1),
                        )

            def emit_vtrans(tcix, v_tiles):
                """copy v psum -> sbuf, PE-transpose per t into V_sb (one wide
                drain copy per chunk)."""
                del ht_tiles[tcix]
                v_sb = vsb_pool.tile([66, CW], FR, tag="v_sb")
                for (lo, hi, var, v_ps) in v_tiles:
                    bal_copy(v_sb[:, lo:hi], v_ps[:, lo:hi], hi - lo)
                vt_ps = peout_pool.tile([128, TL * 66], FR, tag="pe_out")
                for tl in range(TL):
                    nc.tensor.transpose(
                        vt_ps[:, tl * 66:(tl + 1) * 66],
                        v_sb[:, tl * 128:(tl + 1) * 128],
                        ident_sb[0:66, 0:66],
                    )
                bal_copy(V_sb[:, tcix * TL * 66:(tcix + 1) * TL * 66],
                         vt_ps[:], TL * 66)

            # overlap-add assembly in rounds (per watermark) so it overlaps
            # with later chunks instead of serializing at the end
            V3 = V_sb[:].rearrange("b (t m) -> b t m", m=66)
            Y3 = y_sb[:].rearrange("b (j r) -> b j r", r=32)
            B3 = brep_sb[:].rearrange("b (j r) -> b j r", r=32)

            def emit_y_assembly(j_lo, j_hi):
                """Assemble y blocks j in [j_lo, j_hi); requires V[t] for
                t <= j_hi (uses t=j+1 for the r=31 edge). Runs on GpSimd
                (SBUF-only) to keep DVE/ACT free for PSUM drains."""
                eng = nc.gpsimd
                jm = min(j_hi, 32)      # main1 defined for j<=31
                if jm > j_lo:
                    eng.tensor_add(
                        out=Y3[:, j_lo:jm, :], in0=V3[:, j_lo:jm, 1:33],
                        in1=B3[:, j_lo:jm, :])
                if j_hi == 33:          # last block: bias only here
                    eng.tensor_copy(
                        out=y_sb[:, 1024:1056], in_=brep_sb[:, 1024:1056])
                lo = max(1, j_lo)
                if j_hi > lo:           # += v[:, j-1, r+33]
                    eng.tensor_add(
                        out=Y3[:, lo:j_hi, :], in0=Y3[:, lo:j_hi, :],
                        in1=V3[:, lo - 1:j_hi - 1, 33:65])
                lo = max(2, j_lo)
                if j_hi > lo:           # r=0: += v[:, j-2, 65]
                    eng.tensor_add(
                        out=Y3[:, lo:j_hi, 0], in0=Y3[:, lo:j_hi, 0],
                        in1=V3[:, lo - 2:j_hi - 2, 65])
                hi = min(j_hi, 31)
                if hi > j_lo:           # r=31: += v[:, j+1, 0]
                    eng.tensor_add(
                        out=Y3[:, j_lo:hi, 31], in0=Y3[:, j_lo:hi, 31],
                        in1=V3[:, j_lo + 1:hi + 1, 0])

            # rounds: after vtrans(3) -> j<15 (t<=15 avail); after vtrans(6)
            # -> j<27; after vtrans(7) -> all (j<33)
            asm_rounds = {3: (0, 15), 6: (15, 27), 7: (27, 33)}

            # critical path: w1t then the first x piece; everything else after
            nc.sync.dma_start(out=w1t_sb[:], in_=w1t[:])
            # pre-warm the ACT function table during the DMA window so the
            # first relu doesn't pay the ~1.3us LoadActFuncSet
            warm = consts.tile([1, 2], FD)
            nc.gpsimd.memset(warm[:], 0.0)
            nc.scalar.activation(
                warm[:, 1:2], warm[:, 0:1],
                mybir.ActivationFunctionType.Relu, scale=1.0)
            emit_x_load(0, split_first=True)
            nc.sync.dma_start(out=b1c_sb[:], in_=b1c[:])
            GW = HC * C * 3 * 66
            nc.sync.dma_start(out=g_sb[:, 0:GW // 2], in_=g[:, 0:GW // 2])
            emit_x_load(1)
            nc.sync.dma_start(out=g_sb[:, GW // 2:], in_=g[:, GW // 2:])
            nc.sync.dma_start(out=ident_sb[:], in_=ident[:])

            prev = None          # (tcix, v_tiles) awaiting fused stage
            for tcix in range(TC):
                if tcix + 2 < TC:
                    emit_x_load(tcix + 2)
                if tcix == 1:
                    # needed from the first assembly round (after chunk 3)
                    nc.sync.dma_start(out=brep_sb[:], in_=brep[:])
                ht_tiles[tcix] = {
                    (cp, hc): ht_pool.tile(
                        [128, 2 * CW], FH,
                        tag=f"ht{hc}_{cp}", name=f"ht_{tcix}_{hc}_{cp}")
                    for cp in range(C // 2) for hc in range(HC)}
                # interleave: MLP1 half (tcix) with fused half (tcix-1) so PE
                # always has matmul work while relu copies drain PSUM
                for hc in range(HC):
                    emit_mlp1_half(tcix, hc, range(C // 2))
                    if prev is not None:
                        emit_fused_half(prev[0], prev[1], hc)
                if prev is not None:
                    emit_vtrans(prev[0], prev[1])
                    if prev[0] in asm_rounds:
                        emit_y_assembly(*asm_rounds[prev[0]])
                        if prev[0] == 3:
                            # blocks j<15 final: ship the first 480 cols early
                            nc.sync.dma_start(out=y[:, 0:480],
                                              in_=y_sb[:, 0:480])
                        elif prev[0] == 6:
                            nc.sync.dma_start(out=y[:, 480:864],
                                              in_=y_sb[:, 480:864])
                del xt_tiles[tcix]
                v_tiles = [
                    (lo, hi, var,
                     vps_pool.tile([66, CW], FD, tag="v_ps",
                                   name=f"v_ps_{tcix}_{lo}"))
                    for (lo, hi, var) in _chunk_ranges(tcix)]
                prev = (tcix, v_tiles)
            # last chunk: range-major fused so the first ranges' transposes and
            # copies overlap the remaining matmuls, shortening the serial tail
            lt = prev[0]
            ht_last = ht_tiles[lt]
            for (lo, hi, var, v_ps) in prev[1]:
                for hc in range(HC):
                    for c in range(C):
                        i = hc * C + c
                        nc.tensor.matmul(
                            v_ps[:, lo:hi],
                            g_sb[:, _g_col(hc, c, var):_g_col(hc, c, var) + 66],
                            ht_last[(c // 2, hc)][:, (c % 2) * CW + lo:
                                                  (c % 2) * CW + hi],
                            start=(i == 0), stop=(i == HC * C - 1),
                        )
            del ht_tiles[lt]
            v_sb = vsb_pool.tile([66, CW], FR, tag="v_sb")
            vt_ps = peout_pool.tile([128, TL * 66], FR, tag="pe_out")
            for (lo, hi, var, v_ps) in prev[1]:
                bal_copy(v_sb[:, lo:hi], v_ps[:, lo:hi], hi - lo)
                for tl in range(lo // 128, hi // 128):
                    nc.tensor.transpose(
                        vt_ps[:, tl * 66:(tl + 1) * 66],
                        v_sb[:, tl * 128:(tl + 1) * 128],
                        ident_sb[0:66, 0:66],
                    )
            bal_copy(V_sb[:, lt * TL * 66:(lt + 1) * TL * 66],
                     vt_ps[:], TL * 66)
            emit_y_assembly(*asm_rounds[TC - 1])

            nc.sync.dma_start(out=y[:, 864:SIG_LEN], in_=y_sb[:, 864:SIG_LEN])

    nc.finalize()
    return nc


def make_in_maps(inputs):
    """Per-core input maps (shared by kernel(), sim checks, and bench)."""
    x = np.asarray(inputs["encoder_output"], dtype=np.float32)
    W1 = np.asarray(inputs["W1"], np.float32)
    b1 = np.asarray(inputs["b1"], np.float32)

    G, Bvec = _host_prep(
        inputs["W1"], inputs["b1"], inputs["W2"], inputs["b2"],
        inputs["Winv"], inputs["binv"], inputs["Wconv"], inputs["bconv"])

    # pack G -> [128, HC*C*3*66]: g_sb[p, _g_col(hc,c,var)+m] = G[var, c, hc*128+p, m]
    g_pack = np.zeros((128, HC * C * 3 * 66), np.float32)
    for hc in range(HC):
        for c in range(C):
            for var in range(3):
                col = _g_col(hc, c, var)
                g_pack[:, col:col + 66] = G[var, c, hc * 128:(hc + 1) * 128, :]

    w1t = np.ascontiguousarray(W1.T).astype(np.float16)     # [E, H]
    g_pack = g_pack.astype(np.float16)
    b1c = np.ascontiguousarray(b1.reshape(HC, 128).T)       # [128, HC]
    brep = np.ascontiguousarray(np.broadcast_to(Bvec, (BL, SIG_LEN)))
    ident = np.eye(128, dtype=np.float32)

    # [B,C,T,E] -> per-shard [TC, E, C*TL*BL] fp16 (feature-major tokens)
    xh = x.astype(np.float16)
    xs = xh.reshape(N_CORES, BL, C, TC, TL, E).transpose(0, 3, 5, 2, 4, 1)
    xs = np.ascontiguousarray(xs).reshape(N_CORES, TC, E, XW)
    return [
        {
            "x": xs[i],
            "w1t": w1t, "b1c": b1c, "g": g_pack,
            "brep": brep, "ident": ident,
        }
        for i in range(N_CORES)
    ]


def kernel(**inputs) -> np.ndarray:
    if "nc" not in _CACHE:
        _CACHE["nc"] = _build_bass()
    nc = _CACHE["nc"]

    in_maps = make_in_maps(inputs)
    res = run_bass_kernel_spmd(nc, in_maps, core_ids=list(range(N_CORES)))
    _CACHE["last_result"] = res
    y = np.concatenate([r["y"] for r in res.results], axis=0)   # [B, 1056]
    return y.reshape(B, 1, SIG_LEN).astype(np.float32)


if __name__ == "__main__":
    rng = np.random.default_rng(0)
    ins = {
        "encoder_output": rng.standard_normal((B, C, T, E), dtype=np.float32),
        "W1": rng.standard_normal((H, E), dtype=np.float32) / np.sqrt(E),
        "b1": rng.standard_normal((H,), dtype=np.float32) / np.sqrt(E),
        "W2": rng.standard_normal((E, H), dtype=np.float32) / np.sqrt(H),
        "b2": rng.standard_normal((E,), dtype=np.float32) / np.sqrt(H),
        "Winv": rng.standard_normal((SEG_LEN, E), dtype=np.float32) / np.sqrt(E),
        "binv": rng.standard_normal((SEG_LEN,), dtype=np.float32) / np.sqrt(E),
        "Wconv": rng.standard_normal((1, C, 3), dtype=np.float32) / np.sqrt(C * 3),
        "bconv": rng.standard_normal((1,), dtype=np.float32) / np.sqrt(C * 3),
    }
    out = kernel(**ins)
    print("kernel output", out.shape, out.dtype)


# revision 22
# speedup vs baseline: 1.0386x; 1.0386x over previous
"""Trainium2 Bass kernel for nn_Decoder (MLP -> inverse token embedding ->
overlap-add -> channel-merge conv), data-parallel over batch on 8 NeuronCores.

Self-contained: hardcodes shapes; host-side numpy folds everything after the
first Linear+ReLU into per-channel fused matrices G (W2 -> Winv -> overlap-add
normalization -> 3-tap channel conv), and pre-transposes x to feature-major
[TC, E, C*TL*BL] fp16 so the device needs NO transposes of x at all:

    xT[e, tok] --matmul W1T--> h[Hc,tok] in PSUM
    --ACT/DVE relu+bias--> hT in SBUF --matmul G (accum over c,Hc)--> v[66,tok]
    --PE transpose--> vT[b,66] --strided GpSimd adds (overlap-add)--> y[b,1056]

Sharding: batch 1024 -> 8 cores x 128.
"""

import numpy as np

import concourse.bacc as bacc
import concourse.mybir as mybir
from concourse.bass_utils import run_bass_kernel_spmd
from concourse.tile import TileContext

# problem shapes (hardcoded per contract)
B, C, T, E, H = 1024, 8, 32, 128, 256
SEG_LEN, SIG_LEN, NUM_SEG, STEP = 64, 1056, 32, 32
N_CORES = 8
BL = B // N_CORES          # local batch per core = 128
HC = H // 128              # H chunks = 2
TC = 8                     # t-chunks
TL = T // TC               # t per chunk = 4
CW = TL * BL               # tokens per (c, chunk) = 512
XW = C * CW                # tokens per chunk = 4096
FD = mybir.dt.float32
FR = mybir.dt.float32r     # fp32 storage, FP22 multiply
FH = mybir.dt.float16

_CACHE = {}


def _host_prep(W1, b1, W2, b2, Winv, binv, Wconv, bconv):
    """Fold W2/Winv/normalization/conv into G [3var][C][H,66] and bias B[1056]."""
    counter = np.zeros(SIG_LEN, np.float64)
    for t in range(NUM_SEG):
        counter[t * STEP: t * STEP + SEG_LEN] += 1.0
    n = 1.0 / counter

    F = Winv.astype(np.float64) @ W2.astype(np.float64)          # [64, H]
    binv2 = Winv.astype(np.float64) @ b2.astype(np.float64) + binv.astype(np.float64)
    Wc = Wconv[0].astype(np.float64)                             # [C, 3]

    def n_of(var, s):
        if var == 0:
            return n[s]
        if var == 2:
            return n[992 + s]
        return 0.5

    G = np.zeros((3, C, H, 66), np.float64)
    for var in range(3):
        for c in range(C):
            for m_idx in range(66):
                for k in range(3):
                    s = m_idx + k - 2
                    if 0 <= s < SEG_LEN:
                        G[var, c, :, m_idx] += Wc[c, k] * n_of(var, s) * F[s, :]

    sig_b = np.zeros(SIG_LEN, np.float64)
    for t in range(NUM_SEG):
        sig_b[t * STEP: t * STEP + SEG_LEN] += binv2
    sig_b *= n
    Bvec = np.full(SIG_LEN, float(np.asarray(bconv).reshape(-1)[0]), np.float64)
    q = np.arange(SIG_LEN)
    for k in range(3):
        qq = q + k - 1
        valid = (qq >= 0) & (qq < SIG_LEN)
        for c in range(C):
            Bvec[valid] += Wc[c, k] * sig_b[qq[valid]]
    return G.astype(np.float32), Bvec.astype(np.float32)


def _g_col(hc, c, var):
    """Column offset of G slice (hc, c, var) inside g_sb [128, 2*8*3*66]."""
    return ((hc * C + c) * 3 + var) * 66


def _chunk_ranges(tcix):
    # column ranges with uniform G variant; cols = tl*128 + b
    if tcix == 0:
        return [(0, 128, 0), (128, 512, 1)]       # t=0 -> var 0
    if tcix == TC - 1:
        return [(0, 384, 1), (384, 512, 2)]       # t=31 -> var 2
    return [(0, 512, 1)]


def _build_bass():
    nc = bacc.Bacc("TRN2")

    # host pre-transposed: x[tc, e, c*CW + tl*BL + b]  (feature-major)
    x = nc.dram_tensor("x", [TC, E, XW], FH, kind="ExternalInput")
    w1t = nc.dram_tensor("w1t", [E, H], FH, kind="ExternalInput")
    b1c = nc.dram_tensor("b1c", [128, HC], FD, kind="ExternalInput")
    g = nc.dram_tensor("g", [128, HC * C * 3 * 66], FH, kind="ExternalInput")
    brep = nc.dram_tensor("brep", [BL, SIG_LEN], FD, kind="ExternalInput")
    ident = nc.dram_tensor("ident", [128, 128], FR, kind="ExternalInput")
    y = nc.dram_tensor("y", [BL, SIG_LEN], FD, kind="ExternalOutput")

    with TileContext(nc) as tc:
        with (
            tc.tile_pool(name="consts", bufs=1) as consts,
            tc.tile_pool(name="xt", bufs=3) as xt_pool,
            tc.tile_pool(name="ht", bufs=2) as ht_pool,
            tc.tile_pool(name="vsb", bufs=3) as vsb_pool,
            tc.tile_pool(name="big", bufs=1) as big_pool,
            tc.tile_pool(name="h_ps", bufs=5, space="PSUM") as hps_pool,
            tc.tile_pool(name="v_ps", bufs=2, space="PSUM") as vps_pool,
            tc.tile_pool(name="pe_out", bufs=1, space="PSUM") as peout_pool,
        ):
            w1t_sb = consts.tile([E, H], FH)
            b1c_sb = consts.tile([128, HC], FD)
            g_sb = consts.tile([128, HC * C * 3 * 66], FH)
            ident_sb = consts.tile([128, 128], FR)
            brep_sb = big_pool.tile([BL, SIG_LEN], FD)

            V_sb = big_pool.tile([BL, T * 66], FD)      # v transposed: [b, t*66+m]
            y_sb = big_pool.tile([BL, SIG_LEN], FD)

            xt_tiles = {}

            def emit_x_load(tcix, split_first=False):
                t = xt_pool.tile([E, XW], FH, tag="xt", name=f"xt_{tcix}")
                if split_first:
                    # first chunk: land early channels quickly so PE starts sooner
                    nc.sync.dma_start(out=t[:, 0:2 * CW], in_=x[tcix, :, 0:2 * CW])
                    nc.sync.dma_start(out=t[:, 2 * CW:4 * CW],
                                      in_=x[tcix, :, 2 * CW:4 * CW])
                    nc.sync.dma_start(out=t[:, 4 * CW:], in_=x[tcix, :, 4 * CW:])
                else:
                    nc.sync.dma_start(out=t[:], in_=x[tcix])
                xt_tiles[tcix] = t

            # greedy ACT/DVE load balancer for PSUM->SBUF copies and relus
            eng_busy = {"act": 0.0, "dve": 0.0}

            def pick_engine(fd):
                ca = (172 + fd) / 1.2
                cd = (120 + fd) / 0.96
                if eng_busy["act"] + ca <= eng_busy["dve"] + cd:
                    eng_busy["act"] += ca
                    return "act"
                eng_busy["dve"] += cd
                return "dve"

            def bal_copy(out, in_, fd):
                if pick_engine(fd) == "act":
                    nc.scalar.copy(out=out, in_=in_)
                else:
                    nc.vector.tensor_copy(out=out, in_=in_)

            ht_tiles = {}

            def emit_mlp1_half(tcix, hc, cps):
                """one matmul + one relu drain per channel; 5 PSUM bufs keep
                the PE ahead of the ACT/DVE drain latency."""
                xt = xt_tiles[tcix]
                ht = ht_tiles[tcix]
                for cp in cps:
                    for half in range(2):
                        c = 2 * cp + half
                        h_ps = hps_pool.tile([128, CW], FD, tag="h_ps",
                                             name=f"h_ps_{tcix}_{hc}_{c}")
                        nc.tensor.matmul(
                            h_ps[:],
                            w1t_sb[:, hc * 128:(hc + 1) * 128],
                            xt[:, c * CW:(c + 1) * CW],
                            start=True, stop=True,
                        )
                        dst = ht[(cp, hc)][:, half * CW:(half + 1) * CW]
                        if pick_engine(CW) == "act":
                            nc.scalar.activation(
                                dst, h_ps[:],
                                mybir.ActivationFunctionType.Relu,
                                bias=b1c_sb[:, hc:hc + 1], scale=1.0,
                            )
                        else:
                            nc.vector.tensor_scalar(
                                dst, h_ps[:],
                                b1c_sb[:, hc:hc + 1], 0.0,
                                mybir.AluOpType.add, mybir.AluOpType.max,
                            )

            def emit_fused_half(tcix, v_tiles, hc):
                """fused G matmuls (one hc) accumulating into v_tiles ranges."""
                ht = ht_tiles[tcix]
                for c in range(C):
                    i = hc * C + c
                    hsrc = ht[(c // 2, hc)]
                    off = (c % 2) * CW
                    for (lo, hi, var, v_ps) in v_tiles:
                        nc.tensor.matmul(
                            v_ps[:, lo:hi],
                            g_sb[:, _g_col(hc, c, var):_g_col(hc, c, var) + 66],
                            hsrc[:, off + lo:off + hi],
                            start=(i == 0), stop=(i == HC * C - 1),
                        )

            def emit_vtrans(tcix, v_tiles):
                """copy v psum -> sbuf, PE-transpose per t into V_sb (one wide
                drain copy per chunk)."""
                del ht_tiles[tcix]
                v_sb = vsb_pool.tile([66, CW], FR, tag="v_sb")
                for (lo, hi, var, v_ps) in v_tiles:
                    bal_copy(v_sb[:, lo:hi], v_ps[:, lo:hi], hi - lo)
                vt_ps = peout_pool.tile([128, TL * 66], FR, tag="pe_out")
                for tl in range(TL):
                    nc.tensor.transpose(
                        vt_ps[:, tl * 66:(tl + 1) * 66],
                        v_sb[:, tl * 128:(tl + 1) * 128],
                        ident_sb[0:66, 0:66],
                    )
                bal_copy(V_sb[:, tcix * TL * 66:(tcix + 1) * TL * 66],
                         vt_ps[:], TL * 66)

            # overlap-add assembly in rounds (per watermark) so it overlaps
            # with later chunks instead of serializing at the end
            V3 = V_sb[:].rearrange("b (t m) -> b t m", m=66)
            Y3 = y_sb[:].rearrange("b (j r) -> b j r", r=32)
            B3 = brep_sb[:].rearrange("b (j r) -> b j r", r=32)

            def emit_y_assembly(j_lo, j_hi):
                """Assemble y blocks j in [j_lo, j_hi); requires V[t] for
                t <= j_hi (uses t=j+1 for the r=31 edge). Runs on GpSimd
                (SBUF-only) to keep DVE/ACT free for PSUM drains."""
                eng = nc.gpsimd
                jm = min(j_hi, 32)      # main1 defined for j<=31
                if jm > j_lo:
                    eng.tensor_add(
                        out=Y3[:, j_lo:jm, :], in0=V3[:, j_lo:jm, 1:33],
                        in1=B3[:, j_lo:jm, :])
                if j_hi == 33:          # last block: bias only here
                    eng.tensor_copy(
                        out=y_sb[:, 1024:1056], in_=brep_sb[:, 1024:1056])
                lo = max(1, j_lo)
                if j_hi > lo:           # += v[:, j-1, r+33]
                    eng.tensor_add(
                        out=Y3[:, lo:j_hi, :], in0=Y3[:, lo:j_hi, :],
                        in1=V3[:, lo - 1:j_hi - 1, 33:65])
                lo = max(2, j_lo)
                if j_hi > lo:           # r=0: += v[:, j-2, 65]
                    eng.tensor_add(
                        out=Y3[:, lo:j_hi, 0], in0=Y3[:, lo:j_hi, 0],
                        in1=V3[:, lo - 2:j_hi - 2, 65])
                hi = min(j_hi, 31)
                if hi > j_lo:           # r=31: += v[:, j+1, 0]
                    eng.tensor_add(
                        out=Y3[:, j_lo:hi, 31], in0=Y3[:, j_lo:hi, 31],
                        in1=V3[:, j_lo + 1:hi + 1, 0])

            # rounds: after vtrans(3) -> j<15 (t<=15 avail); after vtrans(6)
            # -> j<27; after vtrans(7) -> all (j<33)
            asm_rounds = {3: (0, 15), 6: (15, 27), 7: (27, 33)}

            # critical path: w1t then the first x piece; everything else after
            nc.sync.dma_start(out=w1t_sb[:], in_=w1t[:])
            # pre-warm the ACT function table during the DMA window so the
            # first relu doesn't pay the ~1.3us LoadActFuncSet
            warm = consts.tile([1, 2], FD)
            nc.gpsimd.memset(warm[:], 0.0)
            nc.scalar.activation(
                warm[:, 1:2], warm[:, 0:1],
                mybir.ActivationFunctionType.Relu, scale=1.0)
            emit_x_load(0, split_first=True)
            nc.sync.dma_start(out=b1c_sb[:], in_=b1c[:])
            emit_x_load(1)
            GW = HC * C * 3 * 66
            nc.sync.dma_start(out=g_sb[:, 0:GW // 2], in_=g[:, 0:GW // 2])
            nc.sync.dma_start(out=g_sb[:, GW // 2:], in_=g[:, GW // 2:])
            nc.sync.dma_start(out=ident_sb[:], in_=ident[:])

            prev = None          # (tcix, v_tiles) awaiting fused stage
            for tcix in range(TC):
                if tcix + 2 < TC:
                    emit_x_load(tcix + 2)
                if tcix == 1:
                    # needed from the first assembly round (after chunk 3)
                    nc.sync.dma_start(out=brep_sb[:], in_=brep[:])
                ht_tiles[tcix] = {
                    (cp, hc): ht_pool.tile(
                        [128, 2 * CW], FH,
                        tag=f"ht{hc}_{cp}", name=f"ht_{tcix}_{hc}_{cp}")
                    for cp in range(C // 2) for hc in range(HC)}
                # interleave: MLP1 half (tcix) with fused half (tcix-1) so PE
                # always has matmul work while relu copies drain PSUM
                for hc in range(HC):
                    emit_mlp1_half(tcix, hc, range(C // 2))
                    if prev is not None:
                        emit_fused_half(prev[0], prev[1], hc)
                if prev is not None:
                    emit_vtrans(prev[0], prev[1])
                    if prev[0] in asm_rounds:
                        emit_y_assembly(*asm_rounds[prev[0]])
                        if prev[0] == 3:
                            # blocks j<15 final: ship the first 480 cols early
                            nc.sync.dma_start(out=y[:, 0:480],
                                              in_=y_sb[:, 0:480])
                        elif prev[0] == 6:
                            nc.sync.dma_start(out=y[:, 480:864],
                                              in_=y_sb[:, 480:864])
                del xt_tiles[tcix]
                v_tiles = [
                    (lo, hi, var,
                     vps_pool.tile([66, CW], FD, tag="v_ps",
                                   name=f"v_ps_{tcix}_{lo}"))
                    for (lo, hi, var) in _chunk_ranges(tcix)]
                prev = (tcix, v_tiles)
            # last chunk: range-major fused so the first ranges' transposes and
            # copies overlap the remaining matmuls, shortening the serial tail
            lt = prev[0]
            ht_last = ht_tiles[lt]
            for (lo, hi, var, v_ps) in prev[1]:
                for hc in range(HC):
                    for c in range(C):
                        i = hc * C + c
                        nc.tensor.matmul(
                            v_ps[:, lo:hi],
                            g_sb[:, _g_col(hc, c, var):_g_col(hc, c, var) + 66],
                            ht_last[(c // 2, hc)][:, (c % 2) * CW + lo:
                                                  (c % 2) * CW + hi],
                            start=(i == 0), stop=(i == HC * C - 1),
                        )
            del ht_tiles[lt]
            v_sb = vsb_pool.tile([66, CW], FR, tag="v_sb")
            vt_ps = peout_pool.tile([128, TL * 66], FR, tag="pe_out")
            for (lo, hi, var, v_ps) in prev[1]:
                bal_copy(v_sb[:, lo:hi], v_ps[:, lo:hi], hi - lo)
                for tl in range(lo // 128, hi // 128):
                    nc.tensor.transpose(
                        vt_ps[:, tl * 66:(tl + 1) * 66],
                        v_sb[:, tl * 128:(tl + 1) * 128],
                        ident_sb[0:66, 0:66],
                    )
            bal_copy(V_sb[:, lt * TL * 66:(lt + 1) * TL * 66],
                     vt_ps[:], TL * 66)
            emit_y_assembly(*asm_rounds[TC - 1])

            nc.sync.dma_start(out=y[:, 864:SIG_LEN], in_=y_sb[:, 864:SIG_LEN])

    nc.finalize()
    return nc


def make_in_maps(inputs):
    """Per-core input maps (shared by kernel(), sim checks, and bench)."""
    x = np.asarray(inputs["encoder_output"], dtype=np.float32)
    W1 = np.asarray(inputs["W1"], np.float32)
    b1 = np.asarray(inputs["b1"], np.float32)

    G, Bvec = _host_prep(
        inputs["W1"], inputs["b1"], inputs["W2"], inputs["b2"],
        inputs["Winv"], inputs["binv"], inputs["Wconv"], inputs["bconv"])

    # pack G -> [128, HC*C*3*66]: g_sb[p, _g_col(hc,c,var)+m] = G[var, c, hc*128+p, m]
    g_pack = np.zeros((128, HC * C * 3 * 66), np.float32)
    for hc in range(HC):
        for c in range(C):
            for var in range(3):
                col = _g_col(hc, c, var)
                g_pack[:, col:col + 66] = G[var, c, hc * 128:(hc + 1) * 128, :]

    w1t = np.ascontiguousarray(W1.T).astype(np.float16)     # [E, H]
    g_pack = g_pack.astype(np.float16)
    b1c = np.ascontiguousarray(b1.reshape(HC, 128).T)       # [128, HC]
    brep = np.ascontiguousarray(np.broadcast_to(Bvec, (BL, SIG_LEN)))
    ident = np.eye(128, dtype=np.float32)

    # [B,C,T,E] -> per-shard [TC, E, C*TL*BL] fp16 (feature-major tokens)
    xh = x.astype(np.float16)
    xs = xh.reshape(N_CORES, BL, C, TC, TL, E).transpose(0, 3, 5, 2, 4, 1)
    xs = np.ascontiguousarray(xs).reshape(N_CORES, TC, E, XW)
    return [
        {
            "x": xs[i],
            "w1t": w1t, "b1c": b1c, "g": g_pack,
            "brep": brep, "ident": ident,
        }
        for i in range(N_CORES)
    ]


def kernel(**inputs) -> np.ndarray:
    if "nc" not in _CACHE:
        _CACHE["nc"] = _build_bass()
    nc = _CACHE["nc"]

    in_maps = make_in_maps(inputs)
    res = run_bass_kernel_spmd(nc, in_maps, core_ids=list(range(N_CORES)))
    _CACHE["last_result"] = res
    y = np.concatenate([r["y"] for r in res.results], axis=0)   # [B, 1056]
    return y.reshape(B, 1, SIG_LEN).astype(np.float32)


if __name__ == "__main__":
    rng = np.random.default_rng(0)
    ins = {
        "encoder_output": rng.standard_normal((B, C, T, E), dtype=np.float32),
        "W1": rng.standard_normal((H, E), dtype=np.float32) / np.sqrt(E),
        "b1": rng.standard_normal((H,), dtype=np.float32) / np.sqrt(E),
        "W2": rng.standard_normal((E, H), dtype=np.float32) / np.sqrt(H),
        "b2": rng.standard_normal((E,), dtype=np.float32) / np.sqrt(H),
        "Winv": rng.standard_normal((SEG_LEN, E), dtype=np.float32) / np.sqrt(E),
        "binv": rng.standard_normal((SEG_LEN,), dtype=np.float32) / np.sqrt(E),
        "Wconv": rng.standard_normal((1, C, 3), dtype=np.float32) / np.sqrt(C * 3),
        "bconv": rng.standard_normal((1,), dtype=np.float32) / np.sqrt(C * 3),
    }
    out = kernel(**ins)
    print("kernel output", out.shape, out.dtype)


# revision 25
# speedup vs baseline: 1.0526x; 1.0135x over previous
"""Trainium2 Bass kernel for nn_Decoder (MLP -> inverse token embedding ->
overlap-add -> channel-merge conv), data-parallel over batch on 8 NeuronCores.

Self-contained: hardcodes shapes; host-side numpy folds everything after the
first Linear+ReLU into per-channel fused matrices G (W2 -> Winv -> overlap-add
normalization -> 3-tap channel conv), and pre-transposes x to feature-major
[TC, E, C*TL*BL] fp16 so the device needs NO transposes of x at all:

    xT[e, tok] --matmul W1T--> h[Hc,tok] in PSUM
    --ACT/DVE relu+bias--> hT in SBUF --matmul G (accum over c,Hc)--> v[66,tok]
    --PE transpose--> vT[b,66] --strided GpSimd adds (overlap-add)--> y[b,1056]

Sharding: batch 1024 -> 8 cores x 128.
"""

import numpy as np

import concourse.bacc as bacc
import concourse.mybir as mybir
from concourse.bass_utils import run_bass_kernel_spmd
from concourse.tile import TileContext

# problem shapes (hardcoded per contract)
B, C, T, E, H = 1024, 8, 32, 128, 256
SEG_LEN, SIG_LEN, NUM_SEG, STEP = 64, 1056, 32, 32
N_CORES = 8
BL = B // N_CORES          # local batch per core = 128
HC = H // 128              # H chunks = 2
TC = 8                     # t-chunks
TL = T // TC               # t per chunk = 4
CW = TL * BL               # tokens per (c, chunk) = 512
XW = C * CW                # tokens per chunk = 4096
FD = mybir.dt.float32
FR = mybir.dt.float32r     # fp32 storage, FP22 multiply
FH = mybir.dt.float16

_CACHE = {}


def _host_prep(W1, b1, W2, b2, Winv, binv, Wconv, bconv):
    """Fold W2/Winv/normalization/conv into G [3var][C][H,66] and bias B[1056]."""
    counter = np.zeros(SIG_LEN, np.float64)
    for t in range(NUM_SEG):
        counter[t * STEP: t * STEP + SEG_LEN] += 1.0
    n = 1.0 / counter

    F = Winv.astype(np.float64) @ W2.astype(np.float64)          # [64, H]
    binv2 = Winv.astype(np.float64) @ b2.astype(np.float64) + binv.astype(np.float64)
    Wc = Wconv[0].astype(np.float64)                             # [C, 3]

    def n_of(var, s):
        if var == 0:
            return n[s]
        if var == 2:
            return n[992 + s]
        return 0.5

    G = np.zeros((3, C, H, 66), np.float64)
    for var in range(3):
        for c in range(C):
            for m_idx in range(66):
                for k in range(3):
                    s = m_idx + k - 2
                    if 0 <= s < SEG_LEN:
                        G[var, c, :, m_idx] += Wc[c, k] * n_of(var, s) * F[s, :]

    sig_b = np.zeros(SIG_LEN, np.float64)
    for t in range(NUM_SEG):
        sig_b[t * STEP: t * STEP + SEG_LEN] += binv2
    sig_b *= n
    Bvec = np.full(SIG_LEN, float(np.asarray(bconv).reshape(-1)[0]), np.float64)
    q = np.arange(SIG_LEN)
    for k in range(3):
        qq = q + k - 1
        valid = (qq >= 0) & (qq < SIG_LEN)
        for c in range(C):
            Bvec[valid] += Wc[c, k] * sig_b[qq[valid]]
    return G.astype(np.float32), Bvec.astype(np.float32)


def _g_col(hc, c, var):
    """Column offset of G slice (hc, c, var) inside g_sb [128, 2*8*3*66]."""
    return ((hc * C + c) * 3 + var) * 66


def _chunk_ranges(tcix):
    # column ranges with uniform G variant; cols = tl*128 + b
    if tcix == 0:
        return [(0, 128, 0), (128, 512, 1)]       # t=0 -> var 0
    if tcix == TC - 1:
        return [(0, 384, 1), (384, 512, 2)]       # t=31 -> var 2
    return [(0, 512, 1)]


def _build_bass():
    nc = bacc.Bacc("TRN2")

    # host pre-transposed: x[tc, e, c*CW + tl*BL + b]  (feature-major)
    x = nc.dram_tensor("x", [TC, E, XW], FH, kind="ExternalInput")
    w1t = nc.dram_tensor("w1t", [E, H], FH, kind="ExternalInput")
    b1c = nc.dram_tensor("b1c", [128, HC], FD, kind="ExternalInput")
    g = nc.dram_tensor("g", [128, HC * C * 3 * 66], FH, kind="ExternalInput")
    brep = nc.dram_tensor("brep", [BL, SIG_LEN], FD, kind="ExternalInput")
    ident = nc.dram_tensor("ident", [128, 128], FR, kind="ExternalInput")
    y = nc.dram_tensor("y", [BL, SIG_LEN], FD, kind="ExternalOutput")

    with TileContext(nc) as tc:
        with (
            tc.tile_pool(name="consts", bufs=1) as consts,
            tc.tile_pool(name="xt", bufs=3) as xt_pool,
            tc.tile_pool(name="ht", bufs=2) as ht_pool,
            tc.tile_pool(name="vsb", bufs=3) as vsb_pool,
            tc.tile_pool(name="big", bufs=1) as big_pool,
            tc.tile_pool(name="h_ps", bufs=5, space="PSUM") as hps_pool,
            tc.tile_pool(name="v_ps", bufs=2, space="PSUM") as vps_pool,
            tc.tile_pool(name="pe_out", bufs=1, space="PSUM") as peout_pool,
        ):
            w1t_sb = consts.tile([E, H], FH)
            b1c_sb = consts.tile([128, HC], FD)
            g_sb = consts.tile([128, HC * C * 3 * 66], FH)
            ident_sb = consts.tile([128, 128], FR)
            brep_sb = big_pool.tile([BL, SIG_LEN], FD)

            V_sb = big_pool.tile([BL, T * 66], FD)      # v transposed: [b, t*66+m]
            y_sb = big_pool.tile([BL, SIG_LEN], FD)

            xt_tiles = {}

            def emit_x_load(tcix, split_first=False):
                t = xt_pool.tile([E, XW], FH, tag="xt", name=f"xt_{tcix}")
                if split_first:
                    # first chunk: land early channels quickly so PE starts
                    # sooner; first piece goes via the ACT HWDGE queue so its
                    # descriptor-gen runs parallel to w1t's on sync
                    nc.scalar.dma_start(out=t[:, 0:2 * CW],
                                        in_=x[tcix, :, 0:2 * CW])
                    nc.sync.dma_start(out=t[:, 2 * CW:4 * CW],
                                      in_=x[tcix, :, 2 * CW:4 * CW])
                    nc.sync.dma_start(out=t[:, 4 * CW:], in_=x[tcix, :, 4 * CW:])
                else:
                    nc.sync.dma_start(out=t[:], in_=x[tcix])
                xt_tiles[tcix] = t

            # greedy ACT/DVE load balancer for PSUM->SBUF copies and relus
            eng_busy = {"act": 0.0, "dve": 0.0}

            def pick_engine(fd):
                ca = (172 + fd) / 1.2
                cd = (120 + fd) / 0.96
                if eng_busy["act"] + ca <= eng_busy["dve"] + cd:
                    eng_busy["act"] += ca
                    return "act"
                eng_busy["dve"] += cd
                return "dve"

            def bal_copy(out, in_, fd):
                if pick_engine(fd) == "act":
                    nc.scalar.copy(out=out, in_=in_)
                else:
                    nc.vector.tensor_copy(out=out, in_=in_)

            ht_tiles = {}

            def emit_mlp1_half(tcix, hc, cps):
                """one matmul + one relu drain per channel; 5 PSUM bufs keep
                the PE ahead of the ACT/DVE drain latency."""
                xt = xt_tiles[tcix]
                ht = ht_tiles[tcix]
                for cp in cps:
                    for half in range(2):
                        c = 2 * cp + half
                        h_ps = hps_pool.tile([128, CW], FD, tag="h_ps",
                                             name=f"h_ps_{tcix}_{hc}_{c}")
                        nc.tensor.matmul(
                            h_ps[:],
                            w1t_sb[:, hc * 128:(hc + 1) * 128],
                            xt[:, c * CW:(c + 1) * CW],
                            start=True, stop=True,
                        )
                        dst = ht[(cp, hc)][:, half * CW:(half + 1) * CW]
                        if pick_engine(CW) == "act":
                            nc.scalar.activation(
                                dst, h_ps[:],
                                mybir.ActivationFunctionType.Relu,
                                bias=b1c_sb[:, hc:hc + 1], scale=1.0,
                            )
                        else:
                            nc.vector.tensor_scalar(
                                dst, h_ps[:],
                                b1c_sb[:, hc:hc + 1], 0.0,
                                mybir.AluOpType.add, mybir.AluOpType.max,
                            )

            def emit_fused_half(tcix, v_tiles, hc):
                """fused G matmuls (one hc) accumulating into v_tiles ranges."""
                ht = ht_tiles[tcix]
                for c in range(C):
                    i = hc * C + c
                    hsrc = ht[(c // 2, hc)]
                    off = (c % 2) * CW
                    for (lo, hi, var, v_ps) in v_tiles:
                        nc.tensor.matmul(
                            v_ps[:, lo:hi],
                            g_sb[:, _g_col(hc, c, var):_g_col(hc, c, var) + 66],
                            hsrc[:, off + lo:off + hi],
                            start=(i == 0), stop=(i == HC * C - 1),
                        )

            def emit_vtrans(tcix, v_tiles):
                """copy v psum -> sbuf, PE-transpose per t into V_sb (one wide
                drain copy per chunk)."""
                del ht_tiles[tcix]
                v_sb = vsb_pool.tile([66, CW], FR, tag="v_sb")
                for (lo, hi, var, v_ps) in v_tiles:
                    bal_copy(v_sb[:, lo:hi], v_ps[:, lo:hi], hi - lo)
                vt_ps = peout_pool.tile([128, TL * 66], FR, tag="pe_out")
                for tl in range(TL):
                    nc.tensor.transpose(
                        vt_ps[:, tl * 66:(tl + 1) * 66],
                        v_sb[:, tl * 128:(tl + 1) * 128],
                        ident_sb[0:66, 0:66],
                    )
                bal_copy(V_sb[:, tcix * TL * 66:(tcix + 1) * TL * 66],
                         vt_ps[:], TL * 66)

            # overlap-add assembly in rounds (per watermark) so it overlaps
            # with later chunks instead of serializing at the end
            V3 = V_sb[:].rearrange("b (t m) -> b t m", m=66)
            Y3 = y_sb[:].rearrange("b (j r) -> b j r", r=32)
            B3 = brep_sb[:].rearrange("b (j r) -> b j r", r=32)

            def emit_y_assembly(j_lo, j_hi):
                """Assemble y blocks j in [j_lo, j_hi); requires V[t] for
                t <= j_hi (uses t=j+1 for the r=31 edge). Runs on GpSimd
                (SBUF-only) to keep DVE/ACT free for PSUM drains."""
                eng = nc.gpsimd
                jm = min(j_hi, 32)      # main1 defined for j<=31
                if jm > j_lo:
                    eng.tensor_add(
                        out=Y3[:, j_lo:jm, :], in0=V3[:, j_lo:jm, 1:33],
                        in1=B3[:, j_lo:jm, :])
                if j_hi == 33:          # last block: bias only here
                    eng.tensor_copy(
                        out=y_sb[:, 1024:1056], in_=brep_sb[:, 1024:1056])
                lo = max(1, j_lo)
                if j_hi > lo:           # += v[:, j-1, r+33]
                    eng.tensor_add(
                        out=Y3[:, lo:j_hi, :], in0=Y3[:, lo:j_hi, :],
                        in1=V3[:, lo - 1:j_hi - 1, 33:65])
                lo = max(2, j_lo)
                if j_hi > lo:           # r=0: += v[:, j-2, 65]
                    eng.tensor_add(
                        out=Y3[:, lo:j_hi, 0], in0=Y3[:, lo:j_hi, 0],
                        in1=V3[:, lo - 2:j_hi - 2, 65])
                hi = min(j_hi, 31)
                if hi > j_lo:           # r=31: += v[:, j+1, 0]
                    eng.tensor_add(
                        out=Y3[:, j_lo:hi, 31], in0=Y3[:, j_lo:hi, 31],
                        in1=V3[:, j_lo + 1:hi + 1, 0])

            # rounds: after vtrans(3) -> j<15 (t<=15 avail); after vtrans(6)
            # -> j<27; after vtrans(7) -> all (j<33)
            asm_rounds = {3: (0, 15), 6: (15, 27), 7: (27, 33)}

            # critical path: w1t then the first x piece; everything else after
            nc.sync.dma_start(out=w1t_sb[:], in_=w1t[:])
            emit_x_load(0, split_first=True)
            # pre-warm the ACT function table during the DMA window so the
            # first relu doesn't pay the ~1.3us LoadActFuncSet
            warm = consts.tile([1, 2], FD)
            nc.gpsimd.memset(warm[:], 0.0)
            nc.scalar.activation(
                warm[:, 1:2], warm[:, 0:1],
                mybir.ActivationFunctionType.Relu, scale=1.0)
            nc.sync.dma_start(out=b1c_sb[:], in_=b1c[:])
            emit_x_load(1)
            GW = HC * C * 3 * 66
            nc.sync.dma_start(out=g_sb[:, 0:GW // 2], in_=g[:, 0:GW // 2])
            nc.sync.dma_start(out=g_sb[:, GW // 2:], in_=g[:, GW // 2:])
            nc.sync.dma_start(out=ident_sb[:], in_=ident[:])

            prev = None          # (tcix, v_tiles) awaiting fused stage
            for tcix in range(TC):
                if tcix + 2 < TC:
                    emit_x_load(tcix + 2)
                if tcix == 1:
                    # needed from the first assembly round (after chunk 3)
                    nc.sync.dma_start(out=brep_sb[:], in_=brep[:])
                ht_tiles[tcix] = {
                    (cp, hc): ht_pool.tile(
                        [128, 2 * CW], FH,
                        tag=f"ht{hc}_{cp}", name=f"ht_{tcix}_{hc}_{cp}")
                    for cp in range(C // 2) for hc in range(HC)}
                # interleave: MLP1 half (tcix) with fused half (tcix-1) so PE
                # always has matmul work while relu copies drain PSUM
                for hc in range(HC):
                    emit_mlp1_half(tcix, hc, range(C // 2))
                    if prev is not None:
                        emit_fused_half(prev[0], prev[1], hc)
                if prev is not None:
                    emit_vtrans(prev[0], prev[1])
                    if prev[0] in asm_rounds:
                        emit_y_assembly(*asm_rounds[prev[0]])
                        if prev[0] == 3:
                            # blocks j<15 final: ship the first 480 cols early
                            nc.sync.dma_start(out=y[:, 0:480],
                                              in_=y_sb[:, 0:480])
                        elif prev[0] == 6:
                            nc.sync.dma_start(out=y[:, 480:864],
                                              in_=y_sb[:, 480:864])
                del xt_tiles[tcix]
                v_tiles = [
                    (lo, hi, var,
                     vps_pool.tile([66, CW], FD, tag="v_ps",
                                   name=f"v_ps_{tcix}_{lo}"))
                    for (lo, hi, var) in _chunk_ranges(tcix)]
                prev = (tcix, v_tiles)
            # last chunk: range-major fused so the first ranges' transposes and
            # copies overlap the remaining matmuls, shortening the serial tail
            lt = prev[0]
            ht_last = ht_tiles[lt]
            for (lo, hi, var, v_ps) in prev[1]:
                for hc in range(HC):
                    for c in range(C):
                        i = hc * C + c
                        nc.tensor.matmul(
                            v_ps[:, lo:hi],
                            g_sb[:, _g_col(hc, c, var):_g_col(hc, c, var) + 66],
                            ht_last[(c // 2, hc)][:, (c % 2) * CW + lo:
                                                  (c % 2) * CW + hi],
                            start=(i == 0), stop=(i == HC * C - 1),
                        )
            del ht_tiles[lt]
            v_sb = vsb_pool.tile([66, CW], FR, tag="v_sb")
            vt_ps = peout_pool.tile([128, TL * 66], FR, tag="pe_out")
            for (lo, hi, var, v_ps) in prev[1]:
                bal_copy(v_sb[:, lo:hi], v_ps[:, lo:hi], hi - lo)
                for tl in range(lo // 128, hi // 128):
                    nc.tensor.transpose(
                        vt_ps[:, tl * 66:(tl + 1) * 66],
                        v_sb[:, tl * 128:(tl + 1) * 128],
                        ident_sb[0:66, 0:66],
                    )
            bal_copy(V_sb[:, lt * TL * 66:(lt + 1) * TL * 66],
                     vt_ps[:], TL * 66)
            # final assembly split so the first store's completion latency
            # overlaps the remaining small assembly ops
            emit_y_assembly(27, 31)
            nc.sync.dma_start(out=y[:, 864:992], in_=y_sb[:, 864:992])
            emit_y_assembly(31, 33)
            nc.sync.dma_start(out=y[:, 992:SIG_LEN], in_=y_sb[:, 992:SIG_LEN])

    nc.finalize()
    return nc


def make_in_maps(inputs):
    """Per-core input maps (shared by kernel(), sim checks, and bench)."""
    x = np.asarray(inputs["encoder_output"], dtype=np.float32)
    W1 = np.asarray(inputs["W1"], np.float32)
    b1 = np.asarray(inputs["b1"], np.float32)

    G, Bvec = _host_prep(
        inputs["W1"], inputs["b1"], inputs["W2"], inputs["b2"],
        inputs["Winv"], inputs["binv"], inputs["Wconv"], inputs["bconv"])

    # pack G -> [128, HC*C*3*66]: g_sb[p, _g_col(hc,c,var)+m] = G[var, c, hc*128+p, m]
    g_pack = np.zeros((128, HC * C * 3 * 66), np.float32)
    for hc in range(HC):
        for c in range(C):
            for var in range(3):
                col = _g_col(hc, c, var)
                g_pack[:, col:col + 66] = G[var, c, hc * 128:(hc + 1) * 128, :]

    w1t = np.ascontiguousarray(W1.T).astype(np.float16)     # [E, H]
    g_pack = g_pack.astype(np.float16)
    b1c = np.ascontiguousarray(b1.reshape(HC, 128).T)       # [128, HC]
    brep = np.ascontiguousarray(np.broadcast_to(Bvec, (BL, SIG_LEN)))
    ident = np.eye(128, dtype=np.float32)

    # [B,C,T,E] -> per-shard [TC, E, C*TL*BL] fp16 (feature-major tokens)
    xh = x.astype(np.float16)
    xs = xh.reshape(N_CORES, BL, C, TC, TL, E).transpose(0, 3, 5, 2, 4, 1)
    xs = np.ascontiguousarray(xs).reshape(N_CORES, TC, E, XW)
    return [
        {
            "x": xs[i],
            "w1t": w1t, "b1c": b1c, "g": g_pack,
            "brep": brep, "ident": ident,
        }
        for i in range(N_CORES)
    ]


def kernel(**inputs) -> np.ndarray:
    if "nc" not in _CACHE:
        _CACHE["nc"] = _build_bass()
    nc = _CACHE["nc"]

    in_maps = make_in_maps(inputs)
    res = run_bass_kernel_spmd(nc, in_maps, core_ids=list(range(N_CORES)))
    _CACHE["last_result"] = res
    y = np.concatenate([r["y"] for r in res.results], axis=0)   # [B, 1056]
    return y.reshape(B, 1, SIG_LEN).astype(np.float32)


if __name__ == "__main__":
    rng = np.random.default_rng(0)
    ins = {
        "encoder_output": rng.standard_normal((B, C, T, E), dtype=np.float32),
        "W1": rng.standard_normal((H, E), dtype=np.float32) / np.sqrt(E),
        "b1": rng.standard_normal((H,), dtype=np.float32) / np.sqrt(E),
        "W2": rng.standard_normal((E, H), dtype=np.float32) / np.sqrt(H),
        "b2": rng.standard_normal((E,), dtype=np.float32) / np.sqrt(H),
        "Winv": rng.standard_normal((SEG_LEN, E), dtype=np.float32) / np.sqrt(E),
        "binv": rng.standard_normal((SEG_LEN,), dtype=np.float32) / np.sqrt(E),
        "Wconv": rng.standard_normal((1, C, 3), dtype=np.float32) / np.sqrt(C * 3),
        "bconv": rng.standard_normal((1,), dtype=np.float32) / np.sqrt(C * 3),
    }
    out = kernel(**ins)
    print("kernel output", out.shape, out.dtype)
